# revision 15
# baseline (speedup 1.0000x reference)
"""Distributed Trainium2 kernel for nn_AFMALoss (8 NeuronCores, data-parallel over batch).

Math (per batch b, channel c):
    y_gt    = onehot(target)                          (C,H,W)
    u_gt    = unfold(y_gt, 16)                        (C, 256, 4096)
    u_conv  = unfold(avgpool4x4(y_gt), 16)            (C, 256, 256)
    G       = u_gt^T @ u_conv / 256                   (C, 4096, 256)
    loss    = mean((attentions - G)^2)

Device strategy per core (1 batch each):
  - target is host-permuted to (k, l) "unfold" layout (k = within-patch pixel in
    kappa-order, l = patch index in sigma-order), bf16.  The full-res one-hot
    u_gt (4 x 256 x 4096 per core) is built on-device on VectorE (exact bf16).
  - u_conv (4 x 256 x 256, 0.8%% of the data) is precomputed host-side in the
    same kappa order; all values are multiples of 2^-12, exact in bf16.
  - attentions are host-reordered to sigma row order, two chunks per tile:
    16 contiguous 1 MB DMAs.
  - G tiles are computed by TensorE (bf16 inputs exact, fp32 PSUM accumulate),
    VectorE computes D = A - G (f32 -> bf16), ScalarE computes sum(D^2) via
    Square+accum.  A dummy matmul burst at t=0 lifts the PE HAM throttle.
  - Per-core partial sums are summed on host (mean reduction).
"""

import sys

sys.path.insert(0, "/opt/trn_rl_repo")

import numpy as np
import ml_dtypes

import concourse.bass as bass
import concourse.bacc as bacc
import concourse.mybir as mybir
import concourse.tile as tile
from concourse.tile import add_dep_helper
from concourse.bass_utils import run_bass_kernel_spmd

BF16 = ml_dtypes.bfloat16

B, C, H, W = 8, 4, 1024, 1024
P = 16                      # patch
KK = P * P                  # 256 within-patch pixels
L = (H // P) * (W // P)     # 4096 patches
L2 = 256                    # pooled patches
NCHUNK = 32                 # l-chunks of 128
NPAIR = 16
FD = 2 * C * L2             # 2048 free elems per pair tile
NTOT = float(B * C * L * L2)

_NC_CACHE = {}

# sigma ordering of patches: l = my*256 + dy*64 + mx*4 + dx ; sigma = (dy,dx,my,mx)
_LNAT = np.arange(L).reshape(16, 4, 16, 4)
SIG_OF = np.ascontiguousarray(_LNAT.transpose(1, 3, 0, 2).reshape(L))
# kappa ordering of within-patch pixels: kappa = (gy,gx,k4y,k4x), k=(ky,kx)=(4gy+k4y,4gx+k4x)
_KAP = np.arange(KK)
KNAT = ((_KAP // 64) * 4 + (_KAP // 4) % 4) * 16 + ((_KAP // 16) % 4) * 4 + _KAP % 4


def _build_nc():
    nc = bacc.Bacc(None, target_bir_lowering=False)
    f32 = mybir.dt.float32
    bf16 = mybir.dt.bfloat16

    # tperm quarter-major: [quarter, 256, 1024] so one-hot can start early
    tperm = nc.declare_dram_parameter("tperm", [4, KK, 1024], bf16, isOutput=False)
    # att host-reordered: [chunk, partition(=sigma row), c*m]
    att = nc.declare_dram_parameter("att", [NCHUNK, 128, C * L2], f32, isOutput=False)
    ucvp = nc.declare_dram_parameter("ucv", [C, 2, 128, L2], bf16, isOutput=False)
    out = nc.declare_dram_parameter("out", [1, 1], f32, isOutput=True)

    with tile.TileContext(nc) as tc:
        with (
            tc.tile_pool(name="persist", bufs=1) as pp,
            tc.tile_pool(name="awork", bufs=8) as ap_,
            tc.tile_pool(name="dwork", bufs=3) as dp,
            tc.tile_pool(name="psum_d", bufs=3, space="PSUM") as psd,
            tc.tile_pool(name="psum_w", bufs=1, space="PSUM") as psw,
        ):
            # ---- persistent tiles ----
            tp_sb = [pp.tile([128, L], bf16, name=f"tp{kc}", tag=f"tp{kc}") for kc in range(2)]
            ugt = [
                [pp.tile([128, L], bf16, name=f"ugt{c}_{kc}", tag=f"ugt{c}_{kc}") for kc in range(2)]
                for c in range(C)
            ]
            ucv = [
                [pp.tile([128, L2], bf16, name=f"uc{c}_{kc}", tag=f"uc{c}_{kc}") for kc in range(2)]
                for c in range(C)
            ]
            acc = pp.tile([128, NCHUNK], f32, name="acc", tag="acc")
            acc1 = pp.tile([128, 1], f32, name="acc1", tag="acc1")
            ones = pp.tile([128, 1], f32, name="ones", tag="ones")
            out_sb = pp.tile([1, 1], f32, name="outsb", tag="outsb")
            wz = pp.tile([128, 128], bf16, name="wz", tag="wz")
            rz = pp.tile([128, 512], bf16, name="rz", tag="rz")

            # ---- priority loads: tperm quarters + ucv ----
            # tperm quarter qt covers columns (sigma) [qt*1024, (qt+1)*1024) of
            # both kappa-halves.
            prio_dmas = []
            qtr_dmas = []
            for qt in range(4):
                dk = []
                for kc in range(2):
                    dd = nc.sync.dma_start(
                        tp_sb[kc][:, qt * 1024:(qt + 1) * 1024],
                        tperm[qt, kc * 128:(kc + 1) * 128, :],
                    )
                    dk.append(dd)
                    prio_dmas.append(dd)
                qtr_dmas.append(dk)
            for c in range(C):
                for kc in range(2):
                    prio_dmas.append(nc.sync.dma_start(ucv[c][kc][:], ucvp[c, kc]))

            # ---- PE warm-up burst (lifts HAM throttle before real work) ----
            nc.gpsimd.memset(wz[:], 0.0)
            nc.gpsimd.memset(rz[:], 0.0)
            warm = psw.tile([128, 512], mybir.dt.float32, name="warm", tag="warm")
            NWARM = 18
            for i in range(NWARM):
                nc.tensor.matmul(warm[:], wz[:], rz[:],
                                 start=(i == 0), stop=(i == NWARM - 1))

            # ---- one-hot (VectorE, bf16 4x mode), quarter-major ----
            for qt in range(4):
                cs = slice(qt * 1024, (qt + 1) * 1024)
                for c in range(C):
                    for kc in range(2):
                        nc.vector.tensor_scalar(
                            ugt[c][kc][:, cs], tp_sb[kc][:, cs], float(c), None,
                            mybir.AluOpType.is_equal,
                        )

            # ---- main loop over 32 l-chunks ----
            for q in range(NCHUNK):
                at = ap_.tile([128, C * L2], mybir.dt.float32, name="at", tag="at")
                atd = nc.sync.dma_start(at[:], att[q])
                if q < 6:
                    add_dep_helper(atd.ins, prio_dmas[-1].ins, True, "prio loads first")
                dps = psd.tile([128, C * L2], mybir.dt.float32, name="dps", tag="dps")
                # bank-interleaved order: consecutive MMs target different PSUM
                # banks; each start=True lands only after its bank-sibling's
                # group fully finished (values survive the has_written clear)
                for c, kc in [(0, 0), (2, 0), (0, 1), (2, 1),
                              (1, 0), (3, 0), (1, 1), (3, 1)]:
                    nc.tensor.matmul(
                        dps[:, c * L2:(c + 1) * L2],
                        ugt[c][kc][:, q * 128:(q + 1) * 128],
                        ucv[c][kc][:],
                        start=(kc == 0),
                        stop=(kc == 1),
                    )
                dsb = dp.tile([128, C * L2], bf16, name="dsb", tag="dsb")
                nc.vector.tensor_tensor(
                    dsb[:], at[:], dps[:], op=mybir.AluOpType.subtract
                )
                sq = dp.tile([128, C * L2], bf16, name="sq", tag="sq")
                nc.scalar.activation(
                    sq[:], dsb[:], mybir.ActivationFunctionType.Square,
                    accum_out=acc[:, q:q + 1],
                )

            # ---- final reduce ----
            nc.vector.memset(ones[:], 1.0)
            nc.vector.reduce_sum(acc1[:], acc[:], axis=mybir.AxisListType.X)
            tot = psw.tile([1, 1], mybir.dt.float32, name="tot", tag="warm")
            nc.tensor.matmul(tot[:], acc1[:], ones[:], start=True, stop=True)
            nc.vector.tensor_scalar_mul(out_sb[:], tot[:], 1.0 / NTOT)
            nc.sync.dma_start(out[:], out_sb[:])

    nc.finalize()
    return nc


def _host_prep(target_b):
    """target (1024,1024) int -> (4, 256, 1024) bf16, kappa x sigma, quarter-major."""
    t8 = np.asarray(target_b).reshape(16, 4, 4, 4, 16, 4, 4, 4)
    # axes: (my, dy, gy, k4y, mx, dx, gx, k4x)
    tp = t8.transpose(2, 6, 3, 7, 1, 5, 0, 4).reshape(KK, L)
    tp = np.ascontiguousarray(tp).astype(BF16)
    return np.ascontiguousarray(tp.reshape(KK, 4, 1024).transpose(1, 0, 2))


def _host_att(att_b):
    """(C, L, L2) f32 -> (NCHUNK, 128, C*L2) with rows in sigma order."""
    a = att_b[:, SIG_OF, :]                    # (C, L, L2) rows sigma-ordered
    a = a.transpose(1, 0, 2)                   # (L, C, L2)
    return np.ascontiguousarray(a).reshape(NCHUNK, 128, C * L2)


def _host_ucv(target_b):
    """u_conv scaled by 1/256, kappa row order: (C, 2, 128, L2) bf16 (exact)."""
    t4 = np.asarray(target_b).reshape(256, 4, 256, 4)
    ucs = []
    for c in range(C):
        cnt = (t4 == c).sum(axis=(1, 3), dtype=np.int32)   # pooled counts (256,256)
        uc = cnt.reshape(16, 16, 16, 16).transpose(1, 3, 0, 2).reshape(KK, L2)
        ucs.append(uc[KNAT, :])
    u = np.stack(ucs).astype(np.float32) * (2.0 ** -12)
    return np.ascontiguousarray(u.reshape(C, 2, 128, L2).astype(BF16))


def get_nc():
    if "nc" not in _NC_CACHE:
        _NC_CACHE["nc"] = _build_nc()
    return _NC_CACHE["nc"]


def make_in_maps(target, attentions):
    att = np.asarray(attentions, dtype=np.float32)
    return [
        {
            "tperm": _host_prep(target[b]),
            "att": _host_att(att[b]),
            "ucv": _host_ucv(target[b]),
        }
        for b in range(B)
    ]


def kernel(pred=None, target=None, attentions=None, **kw):
    nc = get_nc()
    in_maps = make_in_maps(target, attentions)
    res = run_bass_kernel_spmd(nc, in_maps, list(range(B)))
    loss = sum(float(r["out"][0, 0]) for r in res.results)
    return np.float32(loss)
